# revision 5
# baseline (speedup 1.0000x reference)
"""BBoxHead (nms_detection) Trainium2 Bass kernel, 8-core SPMD.

Strategy
--------
Data-parallel over the RoI axis: 2000 RoIs -> 250/core (padded to 256).
Activations are kept transposed as [feature, roi] on-chip so that:
  * every weight matrix loads from DRAM in its natural [K, M] layout
    (no transposes anywhere),
  * BatchNorm (training mode, batch stats over ALL 2000 RoIs) reduces along
    the free axis; the cross-core part of the stats is a tiny [128,16] f32
    AllReduce (sum and sum-of-squares per feature),
  * the class/delta heads come out as [roi, class] directly (lhsT = x2n),
    so softmax also reduces along the free axis.
Matmul operands are fp16 (full PE rate, fp32 PSUM accumulation); everything
else (BN, softmax, biases, outputs) is fp32.  The conv biases b1/b2 cancel
exactly in training-mode BN and are skipped.

Host-side prep reorders weights so each SBUF partition receives >=512B
contiguous chunks (two k-rows interleaved per partition).
"""

import numpy as np

import concourse.bass as bass
import concourse.mybir as mybir
import concourse.tile as tile
from concourse import bacc
from concourse import bass_utils

# ---------------------------------------------------------------- constants
N_CORES = 8
N = 2000                      # total RoIs
NLOC = N // N_CORES           # real RoIs per core (250)
NPAD = 256                    # padded RoI columns per core
K1 = 12544                    # 7*7*256 contraction of conv1
HID = 1024
NC = 81
ND = NC * 4                   # 324
NH = NC + ND                  # 405 (heads fused side by side)
EPS = 1e-3
KT1 = K1 // 128               # 98 k-tiles
KT2 = KT1 // 2                # 49 interleaved k-pair tiles
SLAB = 7                      # k-pair tiles per w1 DMA slab (49 = 7*7)
NSLAB = KT2 // SLAB
MT = HID // 128               # 8 feature tiles
F16 = mybir.dt.float16
F32 = mybir.dt.float32

_CACHE = {}


def _build():
    nc = bacc.Bacc("TRN2", target_bir_lowering=False, debug=False,
                   enable_asserts=False, num_devices=N_CORES)

    # ------------------------------------------------ DRAM I/O declarations
    roisT_d = nc.dram_tensor("roisT", [KT2, 128, 2, NPAD], F16, kind="ExternalInput").ap()
    w1_d = nc.dram_tensor("w1t", [MT, KT2, 128, 2, 128], F16, kind="ExternalInput").ap()
    w2_d = nc.dram_tensor("w2t", [MT, 4, 128, 2, 128], F16, kind="ExternalInput").ap()
    wh_d = nc.dram_tensor("wht", [4, 128, 2, NH], F16, kind="ExternalInput").ap()
    bn_d = nc.dram_tensor("bnp", [128, 32], F32, kind="ExternalInput").ap()
    bias_d = nc.dram_tensor("biasb", [128, NH], F32, kind="ExternalInput").ap()

    logits_d = nc.dram_tensor("logits", [NLOC, NC], F32, kind="ExternalOutput").ap()
    probs_d = nc.dram_tensor("probs", [NLOC, NC], F32, kind="ExternalOutput").ap()
    deltas_d = nc.dram_tensor("deltas", [NLOC, ND], F32, kind="ExternalOutput").ap()

    with tile.TileContext(nc) as tc:
        with (
            tc.tile_pool(name="consts", bufs=1) as consts,
            tc.tile_pool(name="w1pool", bufs=4) as w1pool,
            tc.tile_pool(name="acts", bufs=1) as acts,
            tc.tile_pool(name="scr", bufs=2) as scr,
            tc.tile_pool(name="stats", bufs=1) as stats,
            tc.tile_pool(name="ppmm", bufs=3, space="PSUM") as ppmm,
            tc.tile_pool(name="pphd", bufs=2, space="PSUM") as pphd,
            tc.tile_pool(name="dram", bufs=1, space="DRAM") as dram,
        ):
            # ------------------------------------------------ resident loads
            rois_sb = []
            for s in range(NSLAB):
                rt = consts.tile([128, SLAB, 2, NPAD], F16, tag=f"rois{s}")
                nc.sync.dma_start(
                    rt[:], roisT_d[s * SLAB:(s + 1) * SLAB].rearrange("a p b n -> p a b n"))
                rois_sb.append(rt)

            w2_sb = consts.tile([128, MT, 4, 2, 128], F16, tag="w2")
            nc.sync.dma_start(w2_sb[:], w2_d.rearrange("m a p b f -> p m a b f"))
            wh_sb = consts.tile([128, 4, 2, NH], F16, tag="wh")
            nc.sync.dma_start(wh_sb[:], wh_d.rearrange("a p b n -> p a b n"))
            bn_sb = consts.tile([128, 32], F32, tag="bn")
            nc.sync.dma_start(bn_sb[:], bn_d)
            bias_sb = consts.tile([128, NH], F32, tag="biasb")
            nc.sync.dma_start(bias_sb[:], bias_d)
            eps_sb = consts.tile([128, 1], F32, tag="eps")
            nc.vector.memset(eps_sb[:], EPS)

            # --------------------------------------------------- helpers
            def bn_coeffs(gstats, gamma, beta, tag):
                """a = gamma*rsqrt(var+eps); c = beta - mu*a  (all [128, MT])."""
                mu = stats.tile([128, MT], F32, tag=f"mu{tag}")
                ex2 = stats.tile([128, MT], F32, tag=f"ex2{tag}")
                mu2 = stats.tile([128, MT], F32, tag=f"mu2{tag}")
                var = stats.tile([128, MT], F32, tag=f"var{tag}")
                std = stats.tile([128, MT], F32, tag=f"std{tag}")
                rinv = stats.tile([128, MT], F32, tag=f"rinv{tag}")
                a = stats.tile([128, MT], F32, tag=f"a{tag}")
                ac = stats.tile([128, MT], F32, tag=f"ac{tag}")
                c = stats.tile([128, MT], F32, tag=f"c{tag}")
                nc.vector.tensor_scalar_mul(mu[:], gstats[:, 0:MT], 1.0 / N)
                nc.vector.tensor_scalar_mul(ex2[:], gstats[:, MT:2 * MT], 1.0 / N)
                nc.vector.tensor_mul(mu2[:], mu[:], mu[:])
                nc.vector.tensor_sub(var[:], ex2[:], mu2[:])
                nc.scalar.activation(std[:], var[:], mybir.ActivationFunctionType.Sqrt,
                                     bias=eps_sb[:, 0:1])
                nc.vector.reciprocal(rinv[:], std[:])
                nc.vector.tensor_mul(a[:], gamma, rinv[:])
                nc.vector.tensor_mul(ac[:], mu[:], a[:])
                nc.vector.tensor_sub(c[:], beta, ac[:])
                return a, c

            def evict_with_stats(psum, x_sb, st, mt):
                """psum [128,NPAD] f32 -> x_sb; sum/sumsq of first NLOC cols -> st."""
                nc.vector.reduce_sum(st[:, mt:mt + 1], psum[:, :NLOC],
                                     axis=mybir.AxisListType.X)
                sq = scr.tile([128, NLOC], F32, tag="sq")
                nc.scalar.activation(sq[:], psum[:, :NLOC],
                                     mybir.ActivationFunctionType.Square,
                                     accum_out=st[:, MT + mt:MT + mt + 1])
                nc.vector.tensor_copy(x_sb[:], psum[:])

            def allreduce_stats(st, tag):
                ib = dram.tile([128, 2 * MT], F32, tag=f"arin{tag}")
                ob = dram.tile([128, 2 * MT], F32, tag=f"arout{tag}")
                g = stats.tile([128, 2 * MT], F32, tag=f"gst{tag}")
                nc.gpsimd.dma_start(ib[:], st[:])
                nc.gpsimd.collective_compute(
                    "AllReduce", mybir.AluOpType.add,
                    replica_groups=[list(range(N_CORES))],
                    ins=[ib.opt()], outs=[ob.opt()],
                )
                nc.sync.dma_start(g[:], ob[:])
                return g

            # ------------------------------------------------------- layer 1
            stats1 = stats.tile([128, 2 * MT], F32, tag="st1")
            x1_sb = []
            for mt in range(MT):
                psum = ppmm.tile([128, NPAD], F32, tag="mm")
                for s in range(NSLAB):
                    w1t = w1pool.tile([128, SLAB, 2, 128], F16, tag="w1t")
                    nc.sync.dma_start(
                        w1t[:],
                        w1_d[mt, s * SLAB:(s + 1) * SLAB].rearrange("a p b f -> p a b f"))
                    for j in range(SLAB):
                        for k2 in range(2):
                            kt = (s * SLAB + j) * 2 + k2
                            nc.tensor.matmul(psum[:], w1t[:, j, k2, :],
                                             rois_sb[s][:, j, k2, :],
                                             start=(kt == 0), stop=(kt == KT1 - 1))
                xt = acts.tile([128, NPAD], F32, tag=f"x1_{mt}")
                evict_with_stats(psum, xt, stats1, mt)
                x1_sb.append(xt)

            g1 = allreduce_stats(stats1, 1)
            a1, c1 = bn_coeffs(g1, bn_sb[:, 0:MT], bn_sb[:, MT:2 * MT], 1)

            x1n_sb = []
            for mt in range(MT):
                xn = acts.tile([128, NPAD], F16, tag=f"x1n_{mt}")
                nc.scalar.activation(xn[:], x1_sb[mt][:],
                                     mybir.ActivationFunctionType.Relu,
                                     bias=c1[:, mt:mt + 1], scale=a1[:, mt:mt + 1])
                x1n_sb.append(xn)

            # ------------------------------------------------------- layer 2
            stats2 = stats.tile([128, 2 * MT], F32, tag="st2")
            x2_sb = []
            for mt in range(MT):
                psum = ppmm.tile([128, NPAD], F32, tag="mm")
                for kt in range(MT):
                    nc.tensor.matmul(psum[:], w2_sb[:, mt, kt // 2, kt % 2, :],
                                     x1n_sb[kt][:],
                                     start=(kt == 0), stop=(kt == MT - 1))
                xt = acts.tile([128, NPAD], F32, tag=f"x2_{mt}")
                evict_with_stats(psum, xt, stats2, mt)
                x2_sb.append(xt)

            g2 = allreduce_stats(stats2, 2)
            a2, c2 = bn_coeffs(g2, bn_sb[:, 2 * MT:3 * MT], bn_sb[:, 3 * MT:4 * MT], 2)

            x2n_sb = []
            for mt in range(MT):
                xn = acts.tile([128, NPAD], F16, tag=f"x2n_{mt}")
                nc.scalar.activation(xn[:], x2_sb[mt][:],
                                     mybir.ActivationFunctionType.Relu,
                                     bias=c2[:, mt:mt + 1], scale=a2[:, mt:mt + 1])
                x2n_sb.append(xn)

            # --------------------------------------------------------- heads
            for rt in range(2):
                m = 128 if rt == 0 else NLOC - 128
                ph = pphd.tile([128, NH], F32, tag="hd")
                for kt in range(MT):
                    nc.tensor.matmul(ph[:m, :], x2n_sb[kt][:, rt * 128:rt * 128 + m],
                                     wh_sb[:, kt // 2, kt % 2, :],
                                     start=(kt == 0), stop=(kt == MT - 1))
                hb = scr.tile([128, NH], F32, tag="hb")
                nc.vector.tensor_add(hb[:m, :], ph[:m, :], bias_sb[:m, :])
                nc.sync.dma_start(logits_d[rt * 128:rt * 128 + m, :], hb[:m, 0:NC])
                nc.sync.dma_start(deltas_d[rt * 128:rt * 128 + m, :], hb[:m, NC:NH])

                nmx = scr.tile([128, 1], F32, tag="nmx")
                nc.vector.reduce_max(nmx[:m, :], hb[:m, 0:NC],
                                     axis=mybir.AxisListType.X, negate=True)
                ex = scr.tile([128, NC], F32, tag="ex")
                ssum = scr.tile([128, 1], F32, tag="ssum")
                nc.scalar.activation(ex[:m, :], hb[:m, 0:NC],
                                     mybir.ActivationFunctionType.Exp,
                                     bias=nmx[:m, 0:1], accum_out=ssum[:m, 0:1])
                rs = scr.tile([128, 1], F32, tag="rs")
                nc.vector.reciprocal(rs[:m, :], ssum[:m, :])
                pr = scr.tile([128, NC], F32, tag="pr")
                nc.vector.tensor_scalar_mul(pr[:m, :], ex[:m, :], rs[:m, 0:1])
                nc.sync.dma_start(probs_d[rt * 128:rt * 128 + m, :], pr[:m, :])

    nc.compile()
    return nc


def _prep_inputs(pooled_rois, w1, gamma1, beta1, w2, gamma2, beta2,
                 w_logits, b_logits, w_delta, b_delta):
    """Host-side sharding + layout prep. Returns per-core in_maps."""
    rois = np.ascontiguousarray(pooled_rois.reshape(N, K1)).astype(np.float16)
    rois_pad = np.zeros((N_CORES * NPAD, K1), np.float16)
    # place each core's 250 rois at core*NPAD
    for c in range(N_CORES):
        rois_pad[c * NPAD:c * NPAD + NLOC] = rois[c * NLOC:(c + 1) * NLOC]

    w1t = np.ascontiguousarray(
        w1.astype(np.float16).reshape(KT2, 2, 128, MT, 128).transpose(3, 0, 2, 1, 4))
    w2t = np.ascontiguousarray(
        w2.astype(np.float16).reshape(4, 2, 128, MT, 128).transpose(3, 0, 2, 1, 4))
    wh = np.concatenate([w_logits, w_delta], axis=1)  # [1024, 405]
    wht = np.ascontiguousarray(
        wh.astype(np.float16).reshape(4, 2, 128, NH).transpose(0, 2, 1, 3))
    bnp = np.ascontiguousarray(
        np.concatenate([gamma1.reshape(MT, 128).T, beta1.reshape(MT, 128).T,
                        gamma2.reshape(MT, 128).T, beta2.reshape(MT, 128).T],
                       axis=1).astype(np.float32))  # [128, 32]
    biasb = np.ascontiguousarray(
        np.broadcast_to(np.concatenate([b_logits, b_delta]).astype(np.float32),
                        (128, NH)))

    in_maps = []
    for c in range(N_CORES):
        roisT = rois_pad[c * NPAD:(c + 1) * NPAD].T  # [K1, NPAD]
        roisT = np.ascontiguousarray(
            roisT.reshape(KT2, 2, 128, NPAD).transpose(0, 2, 1, 3))
        in_maps.append({
            "roisT": roisT,
            "w1t": w1t, "w2t": w2t, "wht": wht,
            "bnp": bnp, "biasb": biasb,
        })
    return in_maps


def get_nc():
    if "nc" not in _CACHE:
        _CACHE["nc"] = _build()
    return _CACHE["nc"]


def kernel(pooled_rois, w1, b1, gamma1, beta1, w2, b2, gamma2, beta2,
           w_logits, b_logits, w_delta, b_delta):
    # b1/b2 cancel exactly in training-mode batchnorm; unused by design.
    in_maps = _prep_inputs(np.asarray(pooled_rois, np.float32),
                           np.asarray(w1, np.float32),
                           np.asarray(gamma1, np.float32),
                           np.asarray(beta1, np.float32),
                           np.asarray(w2, np.float32),
                           np.asarray(gamma2, np.float32),
                           np.asarray(beta2, np.float32),
                           np.asarray(w_logits, np.float32),
                           np.asarray(b_logits, np.float32),
                           np.asarray(w_delta, np.float32),
                           np.asarray(b_delta, np.float32))
    nc = get_nc()
    res = bass_utils.run_bass_kernel_spmd(
        nc, in_maps, core_ids=list(range(N_CORES)), trace=False)
    logits = np.concatenate([res.results[c]["logits"] for c in range(N_CORES)])
    probs = np.concatenate([res.results[c]["probs"] for c in range(N_CORES)])
    deltas = np.concatenate([res.results[c]["deltas"] for c in range(N_CORES)])
    return logits, probs, deltas.reshape(N, NC, 4)
